# revision 1
# baseline (speedup 1.0000x reference)
"""MQA attention kernel (B=2, T=2048, C=2048, 16 query heads, D=128, RoPE,
causal) for 8 Trainium2 NeuronCores.

Sharding: core = (batch, head-group-of-4). Each core computes q projection for
its 4 heads, the full shared K/V projection for its batch (MQA), causal
attention, and a partial output projection; the host sums the 4 partials per
batch.

Device layout notes:
  - Host pre-transposes x to xT [C, T] so all contractions over C read
    contiguous DRAM.
  - RoPE's even/odd interleave is turned into a half-split layout by permuting
    Wq/Wk columns on the host (scores are invariant to a shared permutation of
    the head dim).  Wq is also pre-scaled by 1/sqrt(D).
  - Scores are computed transposed, S.T[j, i], so the p@V and output
    projections need no on-chip transposes; softmax denominators come from an
    all-ones [128,128] stationary matmul (sums replicated across partitions),
    inverted with a fast-approx reciprocal and fused into the PSUM evacuation.
  - Matmuls run as float32r by default (full PE rate for moving dim >= 256,
    ~4e-4 rel error); KDT=bf16 switches to bf16 (~13% faster, ~6e-3 error).
"""

import os
import sys

if "/opt/trn_rl_repo" not in sys.path:
    sys.path.insert(0, "/opt/trn_rl_repo")

import numpy as np

import concourse.bacc as bacc
import concourse.mybir as mybir
import concourse.tile as tile
from concourse.bass_utils import run_bass_kernel_spmd

T = 2048
C = 2048
D = 128
N_HEAD = 16
HPC = 4  # heads per core
N_CORES = 8
F32 = mybir.dt.float32
F32R = mybir.dt.float32r
BF16 = mybir.dt.bfloat16
EXP = mybir.ActivationFunctionType.Exp


KDT = os.environ.get("KDT", "f32r")


def build_program():
    MD = BF16 if KDT == "bf16" else F32R
    AVD = BF16 if KDT in ("bf16", "mix") else F32R
    nc = bacc.Bacc("TRN2", target_bir_lowering=False, debug=False)

    xt = nc.dram_tensor("xt", [C, T], MD, kind="ExternalInput")
    wq = nc.dram_tensor("wq", [C, HPC * D], MD, kind="ExternalInput")
    wk = nc.dram_tensor("wk", [C, D], MD, kind="ExternalInput")
    wv = nc.dram_tensor("wv", [C, D], MD, kind="ExternalInput")
    wo = nc.dram_tensor("wo", [HPC * D, C], MD, kind="ExternalInput")
    cc = nc.dram_tensor("cc", [D, T], MD, kind="ExternalInput")
    ss = nc.dram_tensor("ss", [D, T], MD, kind="ExternalInput")
    ones_d = nc.dram_tensor("ones_d", [128, 128], AVD, kind="ExternalInput")
    ident_d = nc.dram_tensor("ident_d", [128, 128], MD, kind="ExternalInput")
    out = nc.dram_tensor("out", [T, C], F32, kind="ExternalOutput")

    xt_r = xt.rearrange("(ko p) t -> p ko t", p=128)
    wq_r = wq.rearrange("(ko p) m -> p ko m", p=128)
    wk_r = wk.rearrange("(ko p) m -> p ko m", p=128)
    wv_r = wv.rearrange("(ko p) m -> p ko m", p=128)
    wo_r = wo.rearrange("(ho p) c -> p ho c", p=128)
    out_r = out.rearrange("(mo p) c -> p mo c", p=128)

    with (
        tile.TileContext(nc) as tc,
        tc.tile_pool(name="consts", bufs=1) as consts,
        tc.tile_pool(name="qkpool", bufs=20) as qkpool,
        tc.tile_pool(name="ytpool", bufs=16) as ytpool,
        tc.tile_pool(name="wpool", bufs=1) as wpool,
    ):
        wqs = wpool.tile([128, 16, 512], MD, tag="w")
        for k in range(16):
            eng = [nc.scalar, nc.sync][k % 2]
            eng.dma_start(out=wqs[:, k, :], in_=wq_r[:, k, :])
        ident = consts.tile([128, 128], MD, tag="ident")
        nc.scalar.dma_start(out=ident, in_=ident_d[:, :])
        ones = consts.tile([128, 128], AVD, tag="ones")
        nc.scalar.dma_start(out=ones, in_=ones_d[:, :])
        # tri[j, i] = 1 if i >= j else 0 (keep causal-valid entries)
        tri = consts.tile([128, 128], BF16 if KDT in ("bf16", "mix") else F32, tag="tri")
        nc.gpsimd.memset(tri, 1.0)
        nc.gpsimd.affine_select(
            out=tri,
            in_=tri,
            compare_op=mybir.AluOpType.is_ge,
            fill=0.0,
            base=0,
            pattern=[[1, 128]],
            channel_multiplier=-1,
        )
        ccs = consts.tile([128, T], MD, tag="cc")
        nc.scalar.dma_start(out=ccs, in_=cc[:, :])
        sss = consts.tile([128, T], MD, tag="ss")
        nc.scalar.dma_start(out=sss, in_=ss[:, :])
        wks = consts.tile([128, 16, 128], MD, tag="wk")
        nc.scalar.dma_start(out=wks, in_=wk_r)
        wvs = consts.tile([128, 16, 128], MD, tag="wv")
        nc.scalar.dma_start(out=wvs, in_=wv_r)
        vsb = [
            consts.tile([128, 128], AVD, tag=f"vsb{j}", name=f"vsb{j}")
            for j in range(16)
        ]  # v, natural [t, d] per j-tile

        # qk[idx][c] = 512-wide chunk c of q.T (idx<4) / k.T (idx=4), RoPE'd
        qk = [
            [qkpool.tile([128, 512], MD, tag="qk", name=f"qk{i}_{c}") for c in range(4)]
            for i in range(5)
        ]
        yt = [
            [ytpool.tile([128, 512], MD, tag="yt", name=f"yt{i}_{c}") for c in range(4)]
            for i in range(4)
        ]

        # ---- phase 1: q/k/v projections (contraction over C) ----
        with (
            tc.tile_pool(name="t512", bufs=6) as t512,
            tc.tile_pool(name="psA", bufs=4, space="PSUM") as psA,
            tc.tile_pool(name="psY", bufs=2, space="PSUM") as psY,
            tc.tile_pool(name="psS", bufs=2, space="PSUM") as psS,
        ):
            for tcn in range(4):
                tsl = slice(tcn * 512, (tcn + 1) * 512)
                pq = [psA.tile([128, 512], F32, tag="ps", name=f"pq{i}") for i in range(4)]
                pk = psY.tile([128, 512], F32, tag="py", name=f"pk{tcn}")
                pv = psS.tile([128, 512], F32, tag="pss", name=f"pv{tcn}")
                for k in range(16):
                    xtt = t512.tile([128, 512], MD, tag="xt", name=f"xt{tcn}_{k}")
                    nc.sync.dma_start(out=xtt, in_=xt_r[:, k, tsl])
                    st, sp = k == 0, k == 15
                    for h in range(4):
                        nc.tensor.matmul(
                            pq[h],
                            wqs[:, k, h * 128 : (h + 1) * 128],
                            xtt,
                            start=st,
                            stop=sp,
                        )
                    nc.tensor.matmul(pk, wks[:, k, :], xtt, start=st, stop=sp)
                    nc.tensor.matmul(pv, wvs[:, k, :], xtt, start=st, stop=sp)
                # v natural tiles for this chunk via PE transpose
                vtt = t512.tile([128, 512], MD, tag="misc", name=f"vtt{tcn}")
                nc.scalar.copy(out=vtt, in_=pv)
                for mm in range(4):
                    m = tcn * 4 + mm
                    ptp = psA.tile([128, 512], MD, tag="ps", name=f"ptp{m}")
                    nc.tensor.transpose(
                        ptp[:, :128], vtt[:, mm * 128 : (mm + 1) * 128], ident
                    )
                    nc.scalar.copy(out=vsb[m], in_=ptp[:, :128])
                for h in range(4):
                    nc.scalar.copy(out=qk[h][tcn], in_=pq[h])
                nc.scalar.copy(out=qk[4][tcn], in_=pk)

                # RoPE this chunk (k first so attention unblocks earliest)
                for idx in [4, 0, 1, 2, 3]:
                    qc = qk[idx][tcn]
                    sw = t512.tile([128, 512], MD, tag="sw", name=f"sw{tcn}_{idx}")
                    nc.gpsimd.dma_start(out=sw[0:64, :], in_=qc[64:128, :])
                    nc.gpsimd.dma_start(out=sw[64:128, :], in_=qc[0:64, :])
                    nc.vector.tensor_mul(out=qc[:], in0=qc[:], in1=ccs[:, tsl])
                    nc.gpsimd.tensor_mul(out=sw[:], in0=sw[:], in1=sss[:, tsl])
                    nc.vector.tensor_add(out=qc[:], in0=qc[:], in1=sw[:])

            # load Wo (reuses wq's slot; sync queue is idle after xt)
            wos = wpool.tile([128, 4, T], MD, tag="w")
            nc.sync.dma_start(out=wos, in_=wo_r)

            # ---- phase 2: causal attention, scores transposed S.T[j, i] ----
            for c in range(4):
                for h in range(4):
                    i0 = c * 512
                    py = psY.tile([128, 512], F32, tag="py")
                    psm = psS.tile([128, 512], F32, tag="pss")
                    njj = 4 * c + 4
                    pending = None
                    sums_started = False
                    for jj in range(njj):
                        r = jj - 4 * c  # >= 0 only for diagonal-group tiles
                        off = 128 * r if r >= 0 else 0
                        pss = psA.tile([128, 512], F32, tag="ps")
                        nc.tensor.matmul(
                            pss[:, off:],
                            qk[4][jj // 4][:, (jj % 4) * 128 : (jj % 4 + 1) * 128],
                            qk[h][c][:, off:],
                            start=True,
                            stop=True,
                        )
                        pT = t512.tile([128, 512], AVD, tag="pt")
                        nc.scalar.activation(out=pT[:, off:], in_=pss[:, off:], func=EXP)
                        if r >= 0:
                            nc.vector.tensor_mul(
                                out=pT[:, off : off + 128],
                                in0=pT[:, off : off + 128],
                                in1=tri,
                            )
                        nc.tensor.matmul(
                            py[:, off:],
                            vsb[jj],
                            pT[:, off:],
                            start=jj == 0,
                            stop=jj == njj - 1,
                        )
                        # denominator: pair-sum full tiles on DVE to halve the
                        # extra PE stream; diagonal tiles go individually
                        if r < 0:
                            if pending is None:
                                pending = pT
                            else:
                                pts = t512.tile(
                                    [128, 512], AVD, tag="pts", name=f"pts{c}_{h}_{jj}"
                                )
                                nc.vector.tensor_add(out=pts, in0=pending, in1=pT)
                                nc.tensor.matmul(
                                    psm,
                                    (ones),
                                    pts,
                                    start=not sums_started,
                                    stop=False,
                                )
                                sums_started = True
                                pending = None
                        else:
                            nc.tensor.matmul(
                                psm[:, off:],
                                (ones),
                                (pT[:, off:]),
                                start=not sums_started,
                                stop=jj == njj - 1,
                            )
                            sums_started = True
                    bc = t512.tile([128, 512], F32, tag="misc", name=f"bc{c}_{h}")
                    nc.vector.reciprocal_approx_fast(out=bc, in_=psm)
                    nc.vector.tensor_mul(out=yt[h][c], in0=py, in1=bc)


            # ---- phase 3: partial output projection (contraction over d) ----
            for m in range(16):
                for cn in range(4):
                    po = psA.tile([128, 512], F32, tag="ps")
                    for h in range(4):
                        nc.tensor.matmul(
                            po,
                            yt[h][m // 4][:, (m % 4) * 128 : (m % 4 + 1) * 128],
                            wos[:, h, cn * 512 : (cn + 1) * 512],
                            start=h == 0,
                            stop=h == 3,
                        )
                    ot = t512.tile([128, 512], F32, tag="misc")
                    nc.vector.tensor_copy(out=ot, in_=po)
                    nc.sync.dma_start(out=out_r[:, m, cn * 512 : (cn + 1) * 512], in_=ot)

    nc.compile()
    return nc


_PERM = np.concatenate([np.arange(0, D, 2), np.arange(1, D, 2)])

import ml_dtypes

DT_NP = ml_dtypes.bfloat16 if KDT == "bf16" else np.float32
AV_NP = ml_dtypes.bfloat16 if KDT in ("bf16", "mix") else np.float32


def make_in_maps(x, freqs_cos, freqs_sin, Wq, Wk, Wv, Wo):
    x = np.asarray(x, dtype=np.float32)
    freqs_cos = np.asarray(freqs_cos, dtype=np.float32)
    freqs_sin = np.asarray(freqs_sin, dtype=np.float32)
    Wq = np.asarray(Wq, dtype=np.float32)
    Wk = np.asarray(Wk, dtype=np.float32)
    Wv = np.asarray(Wv, dtype=np.float32)
    Wo = np.asarray(Wo, dtype=np.float32)

    scale = 1.0 / np.sqrt(np.float32(D))
    cosT = np.ascontiguousarray(freqs_cos.T)  # [64, T]
    sinT = np.ascontiguousarray(freqs_sin.T)
    cc = np.ascontiguousarray(np.concatenate([cosT, cosT], axis=0))  # [128, T]
    ss = np.ascontiguousarray(np.concatenate([-sinT, sinT], axis=0))
    wk_p = np.ascontiguousarray(Wk[:, _PERM])
    wv_c = np.ascontiguousarray(Wv)

    xts = [np.ascontiguousarray(x[b].T) for b in range(2)]

    ones_a = np.ones((128, 128), dtype=AV_NP)
    ident_a = np.eye(128, dtype=DT_NP)
    in_maps = []
    for core in range(N_CORES):
        b = core // 4
        hg = core % 4
        heads = range(4 * hg, 4 * hg + 4)
        qcols = np.concatenate([h * D + _PERM for h in heads])
        wq_c = np.ascontiguousarray(Wq[:, qcols] * scale)
        orows = np.concatenate([np.arange(h * D, (h + 1) * D) for h in heads])
        wo_c = np.ascontiguousarray(Wo[orows, :])
        in_maps.append(
            {
                "xt": xts[b].astype(DT_NP),
                "wq": wq_c.astype(DT_NP),
                "wk": wk_p.astype(DT_NP),
                "wv": wv_c.astype(DT_NP),
                "wo": wo_c.astype(DT_NP),
                "cc": cc.astype(DT_NP),
                "ss": ss.astype(DT_NP),
                "ones_d": ones_a,
                "ident_d": ident_a,
            }
        )
    return in_maps


_PROGRAM = None


def get_program():
    global _PROGRAM
    if _PROGRAM is None:
        _PROGRAM = build_program()
    return _PROGRAM


def kernel(x, freqs_cos, freqs_sin, Wq, Wk, Wv, Wo, _collect=None):
    nc = get_program()
    in_maps = make_in_maps(x, freqs_cos, freqs_sin, Wq, Wk, Wv, Wo)
    res = run_bass_kernel_spmd(nc, in_maps, core_ids=list(range(N_CORES)))
    if _collect is not None:
        _collect.append(res)
    outs = [r["out"] for r in res.results]
    full = np.empty((2, T, C), dtype=np.float32)
    for b in range(2):
        full[b] = outs[4 * b] + outs[4 * b + 1] + outs[4 * b + 2] + outs[4 * b + 3]
    return full



# revision 2
# speedup vs baseline: 1.0473x; 1.0473x over previous
"""MQA attention kernel v3 (B=2, T=2048, C=2048, 16 query heads, D=128, RoPE,
causal) for 8 Trainium2 NeuronCores.

Sharding: core = (batch, head-group-of-4), partial output projections summed
on host.

v3 = v2's phase 2/3 (bf16, paired heads, 1024-wide packed exps, quad-summed
denominators) with phase 1 back to t-chunk-major (baseline-style) so that
q/k/v chunks + RoPE finish incrementally and attention can overlap phase 1's
tail instead of waiting for the last head's RoPE chain:
  - per t-chunk: 6 projection outputs accumulate in 3 packed [128,1024] PSUM
    tiles; x streams in [128,512] chunks (fine-grained DMA deps)
  - diagonal-group denominators collapse via shifted DVE adds into a single
    ones-matmul per (head, chunk)
  - DMA order: x chunks + Wk/Wv first, so the PE starts ~9us in (framework
    preamble dominates)
"""

import os
import sys

if "/opt/trn_rl_repo" not in sys.path:
    sys.path.insert(0, "/opt/trn_rl_repo")

import numpy as np

import concourse.bacc as bacc
import concourse.mybir as mybir
import concourse.tile as tile
from concourse.bass_utils import run_bass_kernel_spmd

T = 2048
C = 2048
D = 128
N_HEAD = 16
HPC = 4  # heads per core
N_CORES = 8
F32 = mybir.dt.float32
BF16 = mybir.dt.bfloat16
EXP = mybir.ActivationFunctionType.Exp

MD = BF16  # device dtype for matmul operands


def build_program():
    nc = bacc.Bacc("TRN2", target_bir_lowering=False, debug=False)

    xt = nc.dram_tensor("xt", [C, T], MD, kind="ExternalInput")
    wq = nc.dram_tensor("wq", [C, HPC * D], MD, kind="ExternalInput")
    wk = nc.dram_tensor("wk", [C, D], MD, kind="ExternalInput")
    wv = nc.dram_tensor("wv", [C, D], MD, kind="ExternalInput")
    wo = nc.dram_tensor("wo", [HPC * D, C], MD, kind="ExternalInput")
    cc = nc.dram_tensor("cc", [D, T], MD, kind="ExternalInput")
    ss = nc.dram_tensor("ss", [D, T], MD, kind="ExternalInput")
    ones_d = nc.dram_tensor("ones_d", [128, 128], MD, kind="ExternalInput")
    ident_d = nc.dram_tensor("ident_d", [128, 128], MD, kind="ExternalInput")
    out = nc.dram_tensor("out", [T, C], F32, kind="ExternalOutput")

    xt_r = xt.rearrange("(ko p) t -> p ko t", p=128)
    wq_r = wq.rearrange("(ko p) m -> p ko m", p=128)
    wk_r = wk.rearrange("(ko p) m -> p ko m", p=128)
    wv_r = wv.rearrange("(ko p) m -> p ko m", p=128)
    wo_r = wo.rearrange("(ho p) c -> p ho c", p=128)
    out_r = out.rearrange("(mo p) c -> p mo c", p=128)

    with (
        tile.TileContext(nc) as tc,
        tc.tile_pool(name="xp", bufs=22) as xp,
        tc.tile_pool(name="consts", bufs=1) as consts,
        tc.tile_pool(name="wpool", bufs=1) as wpool,
        tc.tile_pool(name="qkpool", bufs=5) as qkpool,
        tc.tile_pool(name="ytpool", bufs=4) as ytpool,
        tc.tile_pool(name="vttp", bufs=2) as vttp,
        tc.tile_pool(name="ptp", bufs=5) as ptpool,
        tc.tile_pool(name="otp", bufs=3) as otp,
        tc.tile_pool(name="swp", bufs=3) as swp,
        tc.tile_pool(name="pad", bufs=6) as pad,
        tc.tile_pool(name="bcp", bufs=2) as bcp,
        tc.tile_pool(name="ps", bufs=4, space="PSUM") as ps,
    ):
        # ---- input DMAs, earliest-needed first ----
        # tcn0's x chunks and per-kc wq chunks interleave at queue heads so
        # the PE can start ~9us in (right after the framework preamble)
        wks = consts.tile([128, 16, 128], MD, tag="wk")
        nc.scalar.dma_start(out=wks, in_=wk_r)
        wvs = consts.tile([128, 16, 128], MD, tag="wv")
        nc.gpsimd.dma_start(out=wvs, in_=wv_r)
        wqs = wpool.tile([128, 16, 512], MD, tag="w")
        xtt0 = {}
        for kc in range(16):
            xtt = xp.tile([128, 512], MD, tag="xt", name=f"xt0_{kc}")
            [nc.sync, nc.scalar, nc.gpsimd][kc % 3].dma_start(
                out=xtt, in_=xt_r[:, kc, 0:512]
            )
            [nc.scalar, nc.gpsimd, nc.sync][kc % 3].dma_start(
                out=wqs[:, kc, :], in_=wq_r[:, kc, :]
            )
            xtt0[kc] = xtt
        ident = consts.tile([128, 128], MD, tag="ident")
        nc.scalar.dma_start(out=ident, in_=ident_d[:, :])
        ccs = consts.tile([128, T], MD, tag="cc")
        nc.sync.dma_start(out=ccs, in_=cc[:, :])
        sss = consts.tile([128, T], MD, tag="ss")
        nc.scalar.dma_start(out=sss, in_=ss[:, :])
        ones = consts.tile([128, 128], MD, tag="ones")
        nc.scalar.dma_start(out=ones, in_=ones_d[:, :])
        # tri[j, i] = 1 if i >= j else 0 (keep causal-valid entries)
        tri = consts.tile([128, 128], MD, tag="tri")
        nc.gpsimd.memset(tri, 1.0)
        nc.gpsimd.affine_select(
            out=tri,
            in_=tri,
            compare_op=mybir.AluOpType.is_ge,
            fill=0.0,
            base=0,
            pattern=[[1, 128]],
            channel_multiplier=-1,
        )

        # persistent per-head tensors: qk[0..3]=q heads, qk[4]=k (RoPE'd,
        # [d=128, t=2048]); yt[h] = attention output transposed [d, t]
        qk = [qkpool.tile([128, T], MD, tag="qk", name=f"qk{i}") for i in range(5)]
        yt = [ytpool.tile([128, T], MD, tag="yt", name=f"yt{h}") for h in range(4)]
        # v natural layout [t, d] as two [128, 8, 128] groups
        vsb = [consts.tile([128, 8, 128], MD, tag=f"vsb{g}", name=f"vsb{g}") for g in range(2)]

        def vtile(j):
            return vsb[j // 8][:, j % 8, :]

        # ---- phase 1: q/k/v projections, t-chunk-major ----
        for tcn in range(4):
            tsl = slice(512 * tcn, 512 * (tcn + 1))
            pQ = ps.tile([128, 1024], F32, tag="ps", name=f"pQ{tcn}")  # q0|q1
            pR = ps.tile([128, 1024], F32, tag="ps", name=f"pR{tcn}")  # q2|q3
            pKV = ps.tile([128, 1024], F32, tag="ps", name=f"pKV{tcn}")  # k|v
            for kc in range(16):
                if tcn == 0:
                    xtt = xtt0[kc]
                else:
                    xtt = xp.tile([128, 512], MD, tag="xt", name=f"xt{tcn}_{kc}")
                    [nc.sync, nc.scalar, nc.gpsimd][kc % 3].dma_start(
                        out=xtt, in_=xt_r[:, kc, tsl]
                    )
                st, sp = kc == 0, kc == 15
                nc.tensor.matmul(pKV[:, 0:512], wks[:, kc, :], xtt, start=st, stop=sp)
                nc.tensor.matmul(pKV[:, 512:1024], wvs[:, kc, :], xtt, start=st, stop=sp)
                nc.tensor.matmul(pQ[:, 0:512], wqs[:, kc, 0:128], xtt, start=st, stop=sp)
                nc.tensor.matmul(pQ[:, 512:1024], wqs[:, kc, 128:256], xtt, start=st, stop=sp)
                nc.tensor.matmul(pR[:, 0:512], wqs[:, kc, 256:384], xtt, start=st, stop=sp)
                nc.tensor.matmul(pR[:, 512:1024], wqs[:, kc, 384:512], xtt, start=st, stop=sp)
            # evacuate (k first: attention depends on it earliest)
            nc.scalar.copy(out=qk[4][:, tsl], in_=pKV[:, 0:512])
            vtt = vttp.tile([128, 512], MD, tag="vtt", name=f"vtt{tcn}")
            nc.vector.tensor_copy(out=vtt, in_=pKV[:, 512:1024])
            # V transposes into natural layout (4 j-tiles per t-chunk)
            ptp = ps.tile([128, 1024], MD, tag="ps", name=f"ptp{tcn}")
            for mm in range(4):
                nc.tensor.transpose(
                    ptp[:, mm * 128 : (mm + 1) * 128],
                    vtt[:, mm * 128 : (mm + 1) * 128],
                    ident,
                )
            g, r0 = tcn // 2, (tcn % 2) * 4
            nc.vector.tensor_copy(out=vsb[g][:, r0 : r0 + 4, :], in_=ptp[:, 0:512])
            nc.scalar.copy(out=qk[0][:, tsl], in_=pQ[:, 0:512])
            nc.vector.tensor_copy(out=qk[1][:, tsl], in_=pQ[:, 512:1024])
            nc.scalar.copy(out=qk[2][:, tsl], in_=pR[:, 0:512])
            nc.vector.tensor_copy(out=qk[3][:, tsl], in_=pR[:, 512:1024])
            # RoPE per output chunk (k first)
            for o in [4, 0, 1, 2, 3]:
                qc = qk[o]
                sw = swp.tile([128, 512], MD, tag="sw", name=f"sw{tcn}_{o}")
                nc.gpsimd.dma_start(out=sw[0:64, :], in_=qc[64:128, tsl])
                nc.gpsimd.dma_start(out=sw[64:128, :], in_=qc[0:64, tsl])
                nc.vector.tensor_mul(out=qc[:, tsl], in0=qc[:, tsl], in1=ccs[:, tsl])
                eng = nc.gpsimd if o in (1, 2) else nc.vector
                eng.tensor_mul(out=sw[:], in0=sw[:], in1=sss[:, tsl])
                nc.vector.tensor_add(out=qc[:, tsl], in0=qc[:, tsl], in1=sw[:])

        # load Wo into wq's slot
        wos = wpool.tile([128, 4, T], MD, tag="w")
        nc.sync.dma_start(out=wos, in_=wo_r)

        # ---- phase 2: causal attention, scores transposed S.T[j, i] ----
        def ktile(j):
            return qk[4][:, j * 128 : (j + 1) * 128]

        for hp in range(2):
            h0, h1 = 2 * hp, 2 * hp + 1
            for c in range(4):
                qsl = [qk[h][:, c * 512 : (c + 1) * 512] for h in (h0, h1)]
                pyB = ps.tile([128, 1024], F32, tag="ps", name=f"py{hp}_{c}")
                psmB = ps.tile([128, 1024], F32, tag="ps", name=f"psm{hp}_{c}")
                py_on = [False, False]
                sm_on = [False, False]

                # --- off-diagonal tiles, in pairs (j0, j1) ---
                # denominators: pair-sum each pT on DVE, then sum two pairs
                # (quad) before the ones-matmul to halve that PE stream
                pend = [None, None]
                for p in range(2 * c):
                    j0 = 2 * p
                    pss = [
                        ps.tile([128, 1024], F32, tag="ps", name=f"pss{hp}_{c}_{p}_{hi}")
                        for hi in range(2)
                    ]
                    for ji in range(2):
                        kt = ktile(j0 + ji)
                        for hi in range(2):
                            nc.tensor.matmul(
                                pss[hi][:, ji * 512 : (ji + 1) * 512],
                                kt,
                                qsl[hi],
                                start=True,
                                stop=True,
                            )
                    pT = [
                        ptpool.tile([128, 1024], MD, tag="pt", name=f"pt{hp}_{c}_{p}_{hi}")
                        for hi in range(2)
                    ]
                    for hi in range(2):
                        nc.scalar.activation(out=pT[hi], in_=pss[hi], func=EXP)
                    for ji in range(2):
                        vt = vtile(j0 + ji)
                        for hi in range(2):
                            nc.tensor.matmul(
                                pyB[:, hi * 512 : (hi + 1) * 512],
                                vt,
                                pT[hi][:, ji * 512 : (ji + 1) * 512],
                                start=not py_on[hi],
                                stop=False,
                            )
                            py_on[hi] = True
                    for hi in range(2):
                        padd = pad.tile([128, 512], MD, tag="padd", name=f"pa{hp}_{c}_{p}_{hi}")
                        nc.vector.tensor_add(
                            out=padd, in0=pT[hi][:, 0:512], in1=pT[hi][:, 512:1024]
                        )
                        if pend[hi] is None:
                            pend[hi] = padd
                        else:
                            qadd = pad.tile(
                                [128, 512], MD, tag="padd", name=f"qa{hp}_{c}_{p}_{hi}"
                            )
                            nc.vector.tensor_add(out=qadd, in0=pend[hi], in1=padd)
                            nc.tensor.matmul(
                                psmB[:, hi * 512 : (hi + 1) * 512],
                                ones,
                                qadd,
                                start=not sm_on[hi],
                                stop=False,
                            )
                            sm_on[hi] = True
                            pend[hi] = None

                # --- diagonal group: j = 4c+r, r=0..3; packed 896 + 384 ---
                jb = 4 * c
                # tile A: r0 at [0:512] (512 wide), r1 at [512:896] (384 wide)
                pdA = [
                    ps.tile([128, 1024], F32, tag="ps", name=f"pdA{hp}_{c}_{hi}")
                    for hi in range(2)
                ]
                for hi in range(2):
                    nc.tensor.matmul(
                        pdA[hi][:, 0:512], ktile(jb), qsl[hi], start=True, stop=True
                    )
                for hi, h in ((0, h0), (1, h1)):
                    nc.tensor.matmul(
                        pdA[hi][:, 512:896],
                        ktile(jb + 1),
                        qk[h][:, c * 512 + 128 : (c + 1) * 512],
                        start=True,
                        stop=True,
                    )
                pTA = [
                    ptpool.tile([128, 1024], MD, tag="pt", name=f"ptA{hp}_{c}_{hi}")
                    for hi in range(2)
                ]
                for hi in range(2):
                    nc.scalar.activation(
                        out=pTA[hi][:, 0:512], in_=pdA[hi][:, 0:512], func=EXP
                    )
                    nc.vector.tensor_mul(
                        out=pTA[hi][:, 0:128], in0=pTA[hi][:, 0:128], in1=tri
                    )
                for hi in range(2):
                    nc.scalar.activation(
                        out=pTA[hi][:, 512:896], in_=pdA[hi][:, 512:896], func=EXP
                    )
                    nc.vector.tensor_mul(
                        out=pTA[hi][:, 512:640], in0=pTA[hi][:, 512:640], in1=tri
                    )
                # tile B: r2 at [0:256], r3 at [256:384]
                pdB = [
                    ps.tile([128, 1024], F32, tag="ps", name=f"pdB{hp}_{c}_{hi}")
                    for hi in range(2)
                ]
                for hi, h in ((0, h0), (1, h1)):
                    nc.tensor.matmul(
                        pdB[hi][:, 0:256],
                        ktile(jb + 2),
                        qk[h][:, c * 512 + 256 : (c + 1) * 512],
                        start=True,
                        stop=True,
                    )
                for hi, h in ((0, h0), (1, h1)):
                    nc.tensor.matmul(
                        pdB[hi][:, 256:384],
                        ktile(jb + 3),
                        qk[h][:, c * 512 + 384 : (c + 1) * 512],
                        start=True,
                        stop=True,
                    )
                pTB = [
                    ptpool.tile([128, 1024], MD, tag="pt", name=f"ptB{hp}_{c}_{hi}")
                    for hi in range(2)
                ]
                for hi in range(2):
                    nc.scalar.activation(
                        out=pTB[hi][:, 0:384], in_=pdB[hi][:, 0:384], func=EXP
                    )
                    nc.vector.tensor_mul(
                        out=pTB[hi][:, 0:128], in0=pTB[hi][:, 0:128], in1=tri
                    )
                    nc.vector.tensor_mul(
                        out=pTB[hi][:, 256:384], in0=pTB[hi][:, 256:384], in1=tri
                    )
                # PV for the diagonal group (partial query ranges)
                for hi in range(2):
                    nc.tensor.matmul(
                        pyB[:, hi * 512 : hi * 512 + 512],
                        vtile(jb),
                        pTA[hi][:, 0:512],
                        start=not py_on[hi],
                        stop=False,
                    )
                    py_on[hi] = True
                for hi in range(2):
                    nc.tensor.matmul(
                        pyB[:, hi * 512 + 128 : hi * 512 + 512],
                        vtile(jb + 1),
                        pTA[hi][:, 512:896],
                        start=False,
                        stop=False,
                    )
                for hi in range(2):
                    nc.tensor.matmul(
                        pyB[:, hi * 512 + 256 : hi * 512 + 512],
                        vtile(jb + 2),
                        pTB[hi][:, 0:256],
                        start=False,
                        stop=False,
                    )
                for hi in range(2):
                    nc.tensor.matmul(
                        pyB[:, hi * 512 + 384 : hi * 512 + 512],
                        vtile(jb + 3),
                        pTB[hi][:, 256:384],
                        start=False,
                        stop=True,
                    )
                # diagonal denominators: collapse r0..r3 with shifted DVE adds,
                # then one ones-matmul per head
                for hi in range(2):
                    pd = pad.tile([128, 512], MD, tag="padd", name=f"pd{hp}_{c}_{hi}")
                    nc.vector.tensor_copy(out=pd[:, 0:128], in_=pTA[hi][:, 0:128])
                    nc.vector.tensor_add(
                        out=pd[:, 128:512],
                        in0=pTA[hi][:, 128:512],
                        in1=pTA[hi][:, 512:896],
                    )
                    nc.vector.tensor_add(
                        out=pd[:, 256:512], in0=pd[:, 256:512], in1=pTB[hi][:, 0:256]
                    )
                    nc.vector.tensor_add(
                        out=pd[:, 384:512], in0=pd[:, 384:512], in1=pTB[hi][:, 256:384]
                    )
                    nc.tensor.matmul(
                        psmB[:, hi * 512 : hi * 512 + 512],
                        ones,
                        pd,
                        start=not sm_on[hi],
                        stop=True,
                    )
                    sm_on[hi] = True
                # normalize: yt[h][:, c-chunk] = py / sum
                bc = bcp.tile([128, 1024], F32, tag="bc", name=f"bc{hp}_{c}")
                nc.vector.reciprocal_approx_fast(out=bc, in_=psmB)
                for hi, h in ((0, h0), (1, h1)):
                    nc.vector.tensor_mul(
                        out=yt[h][:, c * 512 : (c + 1) * 512],
                        in0=pyB[:, hi * 512 : (hi + 1) * 512],
                        in1=bc[:, hi * 512 : (hi + 1) * 512],
                    )

        # ---- phase 3: partial output projection (contraction over d) ----
        for m in range(16):
            poA = ps.tile([128, 1024], F32, tag="ps", name=f"poA{m}")
            poB = ps.tile([128, 1024], F32, tag="ps", name=f"poB{m}")
            for h in range(4):
                ysl = yt[h][:, m * 128 : (m + 1) * 128]
                st, sp = h == 0, h == 3
                nc.tensor.matmul(poA[:, 0:512], ysl, wos[:, h, 0:512], start=st, stop=sp)
                nc.tensor.matmul(poA[:, 512:1024], ysl, wos[:, h, 512:1024], start=st, stop=sp)
                nc.tensor.matmul(poB[:, 0:512], ysl, wos[:, h, 1024:1536], start=st, stop=sp)
                nc.tensor.matmul(poB[:, 512:1024], ysl, wos[:, h, 1536:2048], start=st, stop=sp)
            otA = otp.tile([128, 1024], F32, tag="ot", name=f"otA{m}")
            nc.scalar.copy(out=otA, in_=poA)
            nc.sync.dma_start(out=out_r[:, m, 0:1024], in_=otA)
            otB = otp.tile([128, 1024], F32, tag="ot", name=f"otB{m}")
            nc.vector.tensor_copy(out=otB, in_=poB)
            nc.scalar.dma_start(out=out_r[:, m, 1024:2048], in_=otB)

    nc.compile()
    return nc


_PERM = np.concatenate([np.arange(0, D, 2), np.arange(1, D, 2)])

import ml_dtypes

DT_NP = ml_dtypes.bfloat16


def make_in_maps(x, freqs_cos, freqs_sin, Wq, Wk, Wv, Wo):
    x = np.asarray(x, dtype=np.float32)
    freqs_cos = np.asarray(freqs_cos, dtype=np.float32)
    freqs_sin = np.asarray(freqs_sin, dtype=np.float32)
    Wq = np.asarray(Wq, dtype=np.float32)
    Wk = np.asarray(Wk, dtype=np.float32)
    Wv = np.asarray(Wv, dtype=np.float32)
    Wo = np.asarray(Wo, dtype=np.float32)

    scale = 1.0 / np.sqrt(np.float32(D))
    cosT = np.ascontiguousarray(freqs_cos.T)  # [64, T]
    sinT = np.ascontiguousarray(freqs_sin.T)
    cc = np.ascontiguousarray(np.concatenate([cosT, cosT], axis=0)).astype(DT_NP)
    ss = np.ascontiguousarray(np.concatenate([-sinT, sinT], axis=0)).astype(DT_NP)
    wk_p = np.ascontiguousarray(Wk[:, _PERM]).astype(DT_NP)
    wv_c = np.ascontiguousarray(Wv).astype(DT_NP)

    xts = [np.ascontiguousarray(x[b].T).astype(DT_NP) for b in range(2)]

    ones_a = np.ones((128, 128), dtype=DT_NP)
    ident_a = np.eye(128, dtype=DT_NP)
    in_maps = []
    for core in range(N_CORES):
        b = core // 4
        hg = core % 4
        heads = range(4 * hg, 4 * hg + 4)
        qcols = np.concatenate([h * D + _PERM for h in heads])
        wq_c = np.ascontiguousarray(Wq[:, qcols] * scale).astype(DT_NP)
        orows = np.concatenate([np.arange(h * D, (h + 1) * D) for h in heads])
        wo_c = np.ascontiguousarray(Wo[orows, :]).astype(DT_NP)
        in_maps.append(
            {
                "xt": xts[b],
                "wq": wq_c,
                "wk": wk_p,
                "wv": wv_c,
                "wo": wo_c,
                "cc": cc,
                "ss": ss,
                "ones_d": ones_a,
                "ident_d": ident_a,
            }
        )
    return in_maps


_PROGRAM = None


def get_program():
    global _PROGRAM
    if _PROGRAM is None:
        _PROGRAM = build_program()
    return _PROGRAM


def kernel(x, freqs_cos, freqs_sin, Wq, Wk, Wv, Wo, _collect=None):
    nc = get_program()
    in_maps = make_in_maps(x, freqs_cos, freqs_sin, Wq, Wk, Wv, Wo)
    res = run_bass_kernel_spmd(nc, in_maps, core_ids=list(range(N_CORES)))
    if _collect is not None:
        _collect.append(res)
    outs = [r["out"] for r in res.results]
    full = np.empty((2, T, C), dtype=np.float32)
    for b in range(2):
        full[b] = outs[4 * b] + outs[4 * b + 1] + outs[4 * b + 2] + outs[4 * b + 3]
    return full


# revision 3
# speedup vs baseline: 1.0514x; 1.0039x over previous
"""MQA attention kernel v4 (B=2, T=2048, C=2048, 16 query heads, D=128, RoPE,
causal) for 8 Trainium2 NeuronCores.

Sharding: core = (batch, head-group-of-4), partial output projections summed
on host.

v4 vs v3: phase 2 processes one head per segment (py/psm take 1 bank each,
score pairs get a 2-deep [128,1024] rotation), and phase 3's output
projection is woven between attention segments in [128,512] quarters that
fill the exp-latency bubbles.  PSUM = big pool 2x[128,1024] (score pairs /
q accumulators) + small pool 4x[128,512] (k/v accumulators, py, psm, po).
Wo's first half gets its own SBUF so early phase-3 quarters never wait on
the Wq-slot reuse.
"""

import os
import sys

if "/opt/trn_rl_repo" not in sys.path:
    sys.path.insert(0, "/opt/trn_rl_repo")

import numpy as np

import concourse.bacc as bacc
import concourse.mybir as mybir
import concourse.tile as tile
from concourse.bass_utils import run_bass_kernel_spmd

T = 2048
C = 2048
D = 128
N_HEAD = 16
HPC = 4
N_CORES = 8
F32 = mybir.dt.float32
BF16 = mybir.dt.bfloat16
EXP = mybir.ActivationFunctionType.Exp

MD = BF16


def build_program():
    nc = bacc.Bacc("TRN2", target_bir_lowering=False, debug=False)

    xt = nc.dram_tensor("xt", [C, T], MD, kind="ExternalInput")
    wq = nc.dram_tensor("wq", [C, HPC * D], MD, kind="ExternalInput")
    wk = nc.dram_tensor("wk", [C, D], MD, kind="ExternalInput")
    wv = nc.dram_tensor("wv", [C, D], MD, kind="ExternalInput")
    wo = nc.dram_tensor("wo", [HPC * D, C], MD, kind="ExternalInput")
    cc = nc.dram_tensor("cc", [D, T], MD, kind="ExternalInput")
    ss = nc.dram_tensor("ss", [D, T], MD, kind="ExternalInput")
    ones_d = nc.dram_tensor("ones_d", [128, 128], MD, kind="ExternalInput")
    ident_d = nc.dram_tensor("ident_d", [128, 128], MD, kind="ExternalInput")
    out = nc.dram_tensor("out", [T, C], F32, kind="ExternalOutput")

    xt_r = xt.rearrange("(ko p) t -> p ko t", p=128)
    wq_r = wq.rearrange("(ko p) m -> p ko m", p=128)
    wk_r = wk.rearrange("(ko p) m -> p ko m", p=128)
    wv_r = wv.rearrange("(ko p) m -> p ko m", p=128)
    wo_r = wo.rearrange("(ho p) c -> p ho c", p=128)
    out_r = out.rearrange("(mo p) c -> p mo c", p=128)

    with (
        tile.TileContext(nc) as tc,
        tc.tile_pool(name="xp", bufs=22) as xp,
        tc.tile_pool(name="consts", bufs=1) as consts,
        tc.tile_pool(name="wpool", bufs=1) as wpool,
        tc.tile_pool(name="woa", bufs=1) as woap,
        tc.tile_pool(name="qkpool", bufs=5) as qkpool,
        tc.tile_pool(name="ytpool", bufs=4) as ytpool,
        tc.tile_pool(name="vttp", bufs=2) as vttp,
        tc.tile_pool(name="ptp", bufs=5) as ptpool,
        tc.tile_pool(name="otp", bufs=4) as otp,
        tc.tile_pool(name="swp", bufs=3) as swp,
        tc.tile_pool(name="pad", bufs=6) as pad,
        tc.tile_pool(name="bcp", bufs=3) as bcp,
        tc.tile_pool(name="psb", bufs=2, space="PSUM") as psb,
        tc.tile_pool(name="pss", bufs=4, space="PSUM") as pssm,
    ):
        # ---- input DMAs, earliest-needed first ----
        wks = consts.tile([128, 16, 128], MD, tag="wk")
        nc.scalar.dma_start(out=wks, in_=wk_r)
        wvs = consts.tile([128, 16, 128], MD, tag="wv")
        nc.gpsimd.dma_start(out=wvs, in_=wv_r)
        wqs = wpool.tile([128, 16, 512], MD, tag="w")
        xtt0 = {}
        for kc in range(16):
            xtt = xp.tile([128, 512], MD, tag="xt", name=f"xt0_{kc}")
            [nc.sync, nc.scalar, nc.gpsimd][kc % 3].dma_start(
                out=xtt, in_=xt_r[:, kc, 0:512]
            )
            [nc.scalar, nc.gpsimd, nc.sync][kc % 3].dma_start(
                out=wqs[:, kc, :], in_=wq_r[:, kc, :]
            )
            xtt0[kc] = xtt
        ident = consts.tile([128, 128], MD, tag="ident")
        nc.scalar.dma_start(out=ident, in_=ident_d[:, :])
        ccs = consts.tile([128, T], MD, tag="cc")
        nc.sync.dma_start(out=ccs, in_=cc[:, :])
        sss = consts.tile([128, T], MD, tag="ss")
        nc.scalar.dma_start(out=sss, in_=ss[:, :])
        ones = consts.tile([128, 128], MD, tag="ones")
        nc.scalar.dma_start(out=ones, in_=ones_d[:, :])
        # first half of Wo in its own slot so early phase-3 work never waits
        woA = woap.tile([128, 4, 1024], MD, tag="woa")
        nc.sync.dma_start(out=woA, in_=wo_r[:, :, 0:1024])
        tri = consts.tile([128, 128], MD, tag="tri")
        nc.gpsimd.memset(tri, 1.0)
        nc.gpsimd.affine_select(
            out=tri,
            in_=tri,
            compare_op=mybir.AluOpType.is_ge,
            fill=0.0,
            base=0,
            pattern=[[1, 128]],
            channel_multiplier=-1,
        )

        qk = [qkpool.tile([128, T], MD, tag="qk", name=f"qk{i}") for i in range(5)]
        yt = [ytpool.tile([128, T], MD, tag="yt", name=f"yt{h}") for h in range(4)]
        vsb = [consts.tile([128, 8, 128], MD, tag=f"vsb{g}", name=f"vsb{g}") for g in range(2)]

        def vtile(j):
            return vsb[j // 8][:, j % 8, :]

        def wosl(h, cn):  # [128, 512] moving slice of Wo for output cols cn
            if cn < 2:
                return woA[:, h, cn * 512 : (cn + 1) * 512]
            return woB[:, h, (cn - 2) * 512 : (cn - 1) * 512]

        def ktile(j):
            return qk[4][:, j * 128 : (j + 1) * 128]

        # ---- phase 2 + 3 woven: attention per (chunk, head); after each
        # chunk's 4 heads, its 4 output-projection m-groups emit as
        # [128,512]-quarter filler ----
        def attn_segment(c, h):
            qsl = qk[h][:, c * 512 : (c + 1) * 512]
            py = pssm.tile([128, 512], F32, tag="small", name=f"py{c}_{h}")
            psm = pssm.tile([128, 512], F32, tag="small", name=f"psm{c}_{h}")
            py_on = False
            sm_on = False
            pend = None
            for p in range(2 * c):
                j0 = 2 * p
                pss = psb.tile([128, 1024], F32, tag="big", name=f"pss{c}_{h}_{p}")
                nc.tensor.matmul(pss[:, 0:512], ktile(j0), qsl, start=True, stop=True)
                nc.tensor.matmul(pss[:, 512:1024], ktile(j0 + 1), qsl, start=True, stop=True)
                pT = ptpool.tile([128, 1024], MD, tag="pt", name=f"pt{c}_{h}_{p}")
                nc.scalar.activation(out=pT, in_=pss, func=EXP)
                nc.tensor.matmul(py, vtile(j0), pT[:, 0:512], start=not py_on, stop=False)
                py_on = True
                nc.tensor.matmul(py, vtile(j0 + 1), pT[:, 512:1024], start=False, stop=False)
                padd = pad.tile([128, 512], MD, tag="padd", name=f"pa{c}_{h}_{p}")
                nc.vector.tensor_add(out=padd, in0=pT[:, 0:512], in1=pT[:, 512:1024])
                if pend is None:
                    pend = padd
                else:
                    qadd = pad.tile([128, 512], MD, tag="padd", name=f"qa{c}_{h}_{p}")
                    nc.vector.tensor_add(out=qadd, in0=pend, in1=padd)
                    nc.tensor.matmul(psm, ones, qadd, start=not sm_on, stop=False)
                    sm_on = True
                    pend = None
            # diagonal group: r0 [0:512] + r1 [512:896] in A; r2 [0:256] +
            # r3 [256:384] in B (both allocated up front: no exp stall)
            jb = 4 * c
            pdA = psb.tile([128, 1024], F32, tag="big", name=f"pdA{c}_{h}")
            pdB = psb.tile([128, 1024], F32, tag="big", name=f"pdB{c}_{h}")
            nc.tensor.matmul(pdA[:, 0:512], ktile(jb), qsl, start=True, stop=True)
            nc.tensor.matmul(
                pdA[:, 512:896],
                ktile(jb + 1),
                qk[h][:, c * 512 + 128 : (c + 1) * 512],
                start=True,
                stop=True,
            )
            nc.tensor.matmul(
                pdB[:, 0:256],
                ktile(jb + 2),
                qk[h][:, c * 512 + 256 : (c + 1) * 512],
                start=True,
                stop=True,
            )
            nc.tensor.matmul(
                pdB[:, 256:384],
                ktile(jb + 3),
                qk[h][:, c * 512 + 384 : (c + 1) * 512],
                start=True,
                stop=True,
            )
            pTA = ptpool.tile([128, 1024], MD, tag="pt", name=f"ptA{c}_{h}")
            pTB = ptpool.tile([128, 1024], MD, tag="pt", name=f"ptB{c}_{h}")
            nc.scalar.activation(out=pTA[:, 0:896], in_=pdA[:, 0:896], func=EXP)
            nc.scalar.activation(out=pTB[:, 0:384], in_=pdB[:, 0:384], func=EXP)
            nc.vector.tensor_mul(out=pTA[:, 0:128], in0=pTA[:, 0:128], in1=tri)
            nc.vector.tensor_mul(out=pTA[:, 512:640], in0=pTA[:, 512:640], in1=tri)
            nc.vector.tensor_mul(out=pTB[:, 0:128], in0=pTB[:, 0:128], in1=tri)
            nc.vector.tensor_mul(out=pTB[:, 256:384], in0=pTB[:, 256:384], in1=tri)
            nc.tensor.matmul(py, vtile(jb), pTA[:, 0:512], start=not py_on, stop=False)
            nc.tensor.matmul(py[:, 128:512], vtile(jb + 1), pTA[:, 512:896], start=False, stop=False)
            nc.tensor.matmul(py[:, 256:512], vtile(jb + 2), pTB[:, 0:256], start=False, stop=False)
            nc.tensor.matmul(py[:, 384:512], vtile(jb + 3), pTB[:, 256:384], start=False, stop=True)
            # diagonal denominators collapse on DVE, then one ones-matmul
            pd = pad.tile([128, 512], MD, tag="padd", name=f"pd{c}_{h}")
            nc.vector.tensor_copy(out=pd[:, 0:128], in_=pTA[:, 0:128])
            nc.vector.tensor_add(out=pd[:, 128:512], in0=pTA[:, 128:512], in1=pTA[:, 512:896])
            nc.vector.tensor_add(out=pd[:, 256:512], in0=pd[:, 256:512], in1=pTB[:, 0:256])
            nc.vector.tensor_add(out=pd[:, 384:512], in0=pd[:, 384:512], in1=pTB[:, 256:384])
            nc.tensor.matmul(psm, ones, pd, start=not sm_on, stop=True)
            bc = bcp.tile([128, 512], F32, tag="bc", name=f"bc{c}_{h}")
            nc.vector.reciprocal_approx_fast(out=bc, in_=psm)
            nc.vector.tensor_mul(
                out=yt[h][:, c * 512 : (c + 1) * 512], in0=py, in1=bc
            )

        def p3_mgroup(m):
            for cn in range(4):
                po = pssm.tile([128, 512], F32, tag="small", name=f"po{m}_{cn}")
                for h in range(4):
                    nc.tensor.matmul(
                        po,
                        yt[h][:, m * 128 : (m + 1) * 128],
                        wosl(h, cn),
                        start=h == 0,
                        stop=h == 3,
                    )
                ot = otp.tile([128, 512], F32, tag="ot", name=f"ot{m}_{cn}")
                if cn % 2 == 0:
                    nc.scalar.copy(out=ot, in_=po)
                    nc.sync.dma_start(out=out_r[:, m, cn * 512 : (cn + 1) * 512], in_=ot)
                else:
                    nc.vector.tensor_copy(out=ot, in_=po)
                    nc.scalar.dma_start(out=out_r[:, m, cn * 512 : (cn + 1) * 512], in_=ot)

        # ---- phase 1: q/k/v projections, t-chunk-major; chunk-0 attention
        # segments (which only need t-chunk 0) weave into the tail ----
        attn_after = {1: [0], 2: [1], 3: [2, 3]}  # tcn -> c0 heads to emit
        for tcn in range(4):
            tsl = slice(512 * tcn, 512 * (tcn + 1))
            pQ = psb.tile([128, 1024], F32, tag="big", name=f"pQ{tcn}")  # q0|q1
            pR = psb.tile([128, 1024], F32, tag="big", name=f"pR{tcn}")  # q2|q3
            pk = pssm.tile([128, 512], F32, tag="small", name=f"pk{tcn}")
            pv = pssm.tile([128, 512], F32, tag="small", name=f"pv{tcn}")
            for kc in range(16):
                if tcn == 0:
                    xtt = xtt0[kc]
                else:
                    xtt = xp.tile([128, 512], MD, tag="xt", name=f"xt{tcn}_{kc}")
                    [nc.sync, nc.scalar, nc.gpsimd][kc % 3].dma_start(
                        out=xtt, in_=xt_r[:, kc, tsl]
                    )
                st, sp = kc == 0, kc == 15
                nc.tensor.matmul(pk, wks[:, kc, :], xtt, start=st, stop=sp)
                nc.tensor.matmul(pv, wvs[:, kc, :], xtt, start=st, stop=sp)
                nc.tensor.matmul(pQ[:, 0:512], wqs[:, kc, 0:128], xtt, start=st, stop=sp)
                nc.tensor.matmul(pQ[:, 512:1024], wqs[:, kc, 128:256], xtt, start=st, stop=sp)
                nc.tensor.matmul(pR[:, 0:512], wqs[:, kc, 256:384], xtt, start=st, stop=sp)
                nc.tensor.matmul(pR[:, 512:1024], wqs[:, kc, 384:512], xtt, start=st, stop=sp)
            # evacuate (k first: attention depends on it earliest)
            nc.scalar.copy(out=qk[4][:, tsl], in_=pk)
            vtt = vttp.tile([128, 512], MD, tag="vtt", name=f"vtt{tcn}")
            nc.vector.tensor_copy(out=vtt, in_=pv)
            ptp = pssm.tile([128, 512], MD, tag="small", name=f"ptp{tcn}")
            for mm in range(4):
                nc.tensor.transpose(
                    ptp[:, mm * 128 : (mm + 1) * 128],
                    vtt[:, mm * 128 : (mm + 1) * 128],
                    ident,
                )
            g, r0 = tcn // 2, (tcn % 2) * 4
            nc.vector.tensor_copy(out=vsb[g][:, r0 : r0 + 4, :], in_=ptp)
            nc.scalar.copy(out=qk[0][:, tsl], in_=pQ[:, 0:512])
            nc.vector.tensor_copy(out=qk[1][:, tsl], in_=pQ[:, 512:1024])
            nc.scalar.copy(out=qk[2][:, tsl], in_=pR[:, 0:512])
            nc.vector.tensor_copy(out=qk[3][:, tsl], in_=pR[:, 512:1024])
            for o in [4, 0, 1, 2, 3]:
                qc = qk[o]
                sw = swp.tile([128, 512], MD, tag="sw", name=f"sw{tcn}_{o}")
                nc.gpsimd.dma_start(out=sw[0:64, :], in_=qc[64:128, tsl])
                nc.gpsimd.dma_start(out=sw[64:128, :], in_=qc[0:64, tsl])
                nc.vector.tensor_mul(out=qc[:, tsl], in0=qc[:, tsl], in1=ccs[:, tsl])
                eng = nc.gpsimd if o in (1, 2) else nc.vector
                eng.tensor_mul(out=sw[:], in0=sw[:], in1=sss[:, tsl])
                nc.vector.tensor_add(out=qc[:, tsl], in0=qc[:, tsl], in1=sw[:])
            for c0h in attn_after.get(tcn, []):
                attn_segment(0, c0h)

        # second half of Wo reuses Wq's slot (free once phase 1 drains)
        woB = wpool.tile([128, 4, 1024], MD, tag="w")
        nc.sync.dma_start(out=woB, in_=wo_r[:, :, 1024:2048])

        for c in range(1, 4):
            for h in range(4):
                attn_segment(c, h)
                p3_mgroup(4 * (c - 1) + h)
        for m in range(12, 16):
            p3_mgroup(m)

    nc.compile()
    return nc


_PERM = np.concatenate([np.arange(0, D, 2), np.arange(1, D, 2)])

import ml_dtypes

DT_NP = ml_dtypes.bfloat16


def make_in_maps(x, freqs_cos, freqs_sin, Wq, Wk, Wv, Wo):
    x = np.asarray(x, dtype=np.float32)
    freqs_cos = np.asarray(freqs_cos, dtype=np.float32)
    freqs_sin = np.asarray(freqs_sin, dtype=np.float32)
    Wq = np.asarray(Wq, dtype=np.float32)
    Wk = np.asarray(Wk, dtype=np.float32)
    Wv = np.asarray(Wv, dtype=np.float32)
    Wo = np.asarray(Wo, dtype=np.float32)

    scale = 1.0 / np.sqrt(np.float32(D))
    cosT = np.ascontiguousarray(freqs_cos.T)
    sinT = np.ascontiguousarray(freqs_sin.T)
    cc = np.ascontiguousarray(np.concatenate([cosT, cosT], axis=0)).astype(DT_NP)
    ss = np.ascontiguousarray(np.concatenate([-sinT, sinT], axis=0)).astype(DT_NP)
    wk_p = np.ascontiguousarray(Wk[:, _PERM]).astype(DT_NP)
    wv_c = np.ascontiguousarray(Wv).astype(DT_NP)

    xts = [np.ascontiguousarray(x[b].T).astype(DT_NP) for b in range(2)]

    ones_a = np.ones((128, 128), dtype=DT_NP)
    ident_a = np.eye(128, dtype=DT_NP)
    in_maps = []
    for core in range(N_CORES):
        b = core // 4
        hg = core % 4
        heads = range(4 * hg, 4 * hg + 4)
        qcols = np.concatenate([h * D + _PERM for h in heads])
        wq_c = np.ascontiguousarray(Wq[:, qcols] * scale).astype(DT_NP)
        orows = np.concatenate([np.arange(h * D, (h + 1) * D) for h in heads])
        wo_c = np.ascontiguousarray(Wo[orows, :]).astype(DT_NP)
        in_maps.append(
            {
                "xt": xts[b],
                "wq": wq_c,
                "wk": wk_p,
                "wv": wv_c,
                "wo": wo_c,
                "cc": cc,
                "ss": ss,
                "ones_d": ones_a,
                "ident_d": ident_a,
            }
        )
    return in_maps


_PROGRAM = None


def get_program():
    global _PROGRAM
    if _PROGRAM is None:
        _PROGRAM = build_program()
    return _PROGRAM


def kernel(x, freqs_cos, freqs_sin, Wq, Wk, Wv, Wo, _collect=None):
    nc = get_program()
    in_maps = make_in_maps(x, freqs_cos, freqs_sin, Wq, Wk, Wv, Wo)
    res = run_bass_kernel_spmd(nc, in_maps, core_ids=list(range(N_CORES)))
    if _collect is not None:
        _collect.append(res)
    outs = [r["out"] for r in res.results]
    full = np.empty((2, T, C), dtype=np.float32)
    for b in range(2):
        full[b] = outs[4 * b] + outs[4 * b + 1] + outs[4 * b + 2] + outs[4 * b + 3]
    return full
